# revision 4
# baseline (speedup 1.0000x reference)
"""Causal self-attention (GQA + partial RoPE + q_gain) Trainium2 Bass kernel.

Model: B=4, T=2048, D=2048, H=16 q-heads, Hkv=4 kv-heads, hD=128, ROPE=64.
Sharding: 8 cores = 4 batches x 2 head-halves (heads h*8..h*8+7, kv heads 2h, 2h+1).
Wq/Wkv column-sharded, Wo row-sharded; host sums the two partial outputs per batch.

Per-core dataflow (all matmuls fp32r = full PE rate, fp32 storage):
  phase A: projections. Q.T/K.T in [channel, token] layout (W stationary, x.T moving),
           V in natural [token, channel] layout (x.T stationary, Wv.T moving).
           Partial-RoPE via "swap projection": an extra projection of the partner
           rope channels makes RoPE 3 partition-aligned elementwise ops on DVE
           (cross-partition combines are illegal on DVE).
           q_gain and 1/sqrt(hD) are folded into Wq host-side (rotation commutes
           with per-head scalars).
  phase B: attention per (head, 512-wide i-block). Scores computed TRANSPOSED
           S.T[s, i] (K.T stationary, Q.T moving) so that P.T = exp(S.T) feeds the
           AV matmul directly (V natural stationary, P.T moving) -> O.T[c, i].
           Causal masking: additive -1e9 masks accumulated into the scores PSUM via
           an identity-stationary matmul (diagonal tiles only); exp of masked = 0.
           Softmax denominators D[i] = colsums of P.T via ones-stationary (M=1)
           matmuls accumulated alongside AV; 1/D replicated to 128 partitions via a
           K=1 ones matmul; O.T normalized during the PSUM->SBUF copy.
  phase C: output projection, TRANSPOSED: out.T[dout, t] (Wo.T-tile stationary,
           O.T moving). Host adds the two partials per batch and transposes back.
"""
import numpy as np

import concourse.bass as bass
import concourse.tile as tile
from concourse import bacc, mybir
from concourse.bass_utils import run_bass_kernel_spmd
from contextlib import ExitStack

F32 = mybir.dt.float32
F32R = mybir.dt.float32r
AF = mybir.ActivationFunctionType
AOp = mybir.AluOpType

B, T, D = 4, 2048, 2048
H, Hkv = 16, 4
hD = 128
ROPE = 64
NB = T // 512          # 4 blocks of 512 tokens
HL = H // 2            # 8 heads per core
GL = Hkv // 2          # 2 kv heads per core


def build_nc():
    nc = bacc.Bacc(trn_type="TRN2", target_bir_lowering=False, debug=False)
    xT = nc.dram_tensor("xT", [D, T], F32R, kind="ExternalInput").ap()
    wqT = nc.dram_tensor("wqT", [D, HL * hD], F32R, kind="ExternalInput").ap()
    wqswT = nc.dram_tensor("wqswT", [D, HL * ROPE], F32R, kind="ExternalInput").ap()
    wkT = nc.dram_tensor("wkT", [D, GL * hD], F32R, kind="ExternalInput").ap()
    wkswT = nc.dram_tensor("wkswT", [D, GL * ROPE], F32R, kind="ExternalInput").ap()
    wvT = nc.dram_tensor("wvT", [D, GL * hD], F32R, kind="ExternalInput").ap()
    woT = nc.dram_tensor("woT", [HL * hD, D], F32R, kind="ExternalInput").ap()
    cosb = nc.dram_tensor("cosb", [ROPE, T], F32R, kind="ExternalInput").ap()
    sinb = nc.dram_tensor("sinb", [2 * ROPE, T], F32R, kind="ExternalInput").ap()
    maskb = nc.dram_tensor("maskb", [128, 4, 512], F32R, kind="ExternalInput").ap()
    ident = nc.dram_tensor("ident", [128, 128], F32R, kind="ExternalInput").ap()
    ones_c = nc.dram_tensor("ones_c", [128, 1], F32R, kind="ExternalInput").ap()
    ones_r = nc.dram_tensor("ones_r", [1, 128], F32R, kind="ExternalInput").ap()
    outT = nc.dram_tensor("outT", [D, T], F32, kind="ExternalOutput").ap()

    with tile.TileContext(nc) as tc, ExitStack() as ctx:
        const = ctx.enter_context(tc.tile_pool(name="const", bufs=1))
        persist = ctx.enter_context(tc.tile_pool(name="persist", bufs=1))

        tid = const.tile([128, 128], F32R, tag="tid")
        nc.sync.dma_start(tid[:], ident)
        toc = const.tile([128, 1], F32R, tag="toc")
        nc.sync.dma_start(toc[:], ones_c)
        tor = const.tile([1, 128], F32R, tag="tor")
        nc.sync.dma_start(tor[:], ones_r)
        tmask = const.tile([128, 4, 512], F32R, tag="tmask")
        nc.sync.dma_start(tmask[:], maskb)

        dramp = ctx.enter_context(tc.tile_pool(name="dramp", bufs=1, space="DRAM"))
        QTd = dramp.tile([128, HL, T], F32R, tag="QTd")
        KT = persist.tile([128, GL, T], F32R, tag="KT")
        VT = persist.tile([128, T // 128, GL * hD], F32R, tag="VT")
        OT = persist.tile([128, HL, T], F32R, tag="OT")

        # ---------------- phase A: projections + rope ----------------
        with ExitStack() as actx:
            tabs = actx.enter_context(tc.tile_pool(name="tabs", bufs=1))
            xpool = actx.enter_context(tc.tile_pool(name="xp", bufs=16))
            wpool = actx.enter_context(tc.tile_pool(name="wp", bufs=3))
            swpool = actx.enter_context(tc.tile_pool(name="swp", bufs=2))
            rtmp = actx.enter_context(tc.tile_pool(name="rtmp", bufs=2))
            qblkp = actx.enter_context(tc.tile_pool(name="qblkp", bufs=3))
            psA = actx.enter_context(tc.tile_pool(name="psA", bufs=1, space="PSUM"))

            tcos = tabs.tile([ROPE, T], F32R, tag="tcos")
            nc.sync.dma_start(tcos[:], cosb)
            tsin = tabs.tile([2 * ROPE, T], F32R, tag="tsin")
            nc.sync.dma_start(tsin[:], sinb)

            for tb in range(NB):
                tsl = slice(512 * tb, 512 * (tb + 1))
                xts = []
                for d in range(16):
                    xt = xpool.tile([128, 512], F32R, tag="xt", name=f"xt{tb}_{d}")
                    nc.sync.dma_start(xt[:], xT[128 * d:128 * (d + 1), tsl])
                    xts.append(xt)

                # K projection -> KT[:, g, tsl]  (weights streamed per use)
                for g in range(GL):
                    pk = psA.tile([128, 512], F32, tag="pk", bufs=2, name=f"pk{tb}_{g}")
                    for d in range(16):
                        wk = wpool.tile([128, 128], F32R, tag="wk", name=f"wk{tb}_{g}_{d}")
                        nc.sync.dma_start(wk[:], wkT[128 * d:128 * (d + 1), 128 * g:128 * (g + 1)])
                        nc.tensor.matmul(pk[:], wk[:], xts[d][:], start=(d == 0), stop=(d == 15))
                    nc.vector.tensor_copy(KT[:][:, g, tsl], pk[:])
                # Ksw projection + K rope
                pksw = psA.tile([128, 512], F32, tag="pksw", bufs=1, name=f"pksw{tb}")
                for d in range(16):
                    wksw = wpool.tile([128, GL * ROPE], F32R, tag="wksw", name=f"wksw{tb}_{d}")
                    nc.sync.dma_start(wksw[:], wkswT[128 * d:128 * (d + 1), :])
                    nc.tensor.matmul(pksw[:], wksw[:], xts[d][:], start=(d == 0), stop=(d == 15))
                ksw = swpool.tile([128, 512], F32R, tag="ksw", name=f"ksw{tb}")
                nc.vector.tensor_copy(ksw[:], pksw[:])
                for g in range(GL):
                    b0 = ROPE * g
                    ts_ = rtmp.tile([ROPE, 512], F32R, tag="ts_", name=f"kts{tb}_{g}")
                    tc_ = rtmp.tile([ROPE, 512], F32R, tag="tc_", name=f"ktc{tb}_{g}")
                    nc.vector.tensor_mul(ts_[:], ksw[b0:b0 + ROPE, :], tsin[b0:b0 + ROPE, tsl])
                    nc.vector.tensor_mul(tc_[:], KT[:][0:ROPE, g, tsl], tcos[:, tsl])
                    nc.vector.tensor_tensor(out=KT[:][0:ROPE, g, tsl], in0=tc_[:], in1=ts_[:], op=AOp.add)
                # V projection (natural layout): VT[:, 4*tb+tt, :], tt-groups of 2
                for ttg in range(2):
                    pvs = []
                    for tt in range(2):
                        pv = psA.tile([128, GL * hD], F32, tag=f"pv{tt}", bufs=1,
                                      name=f"pv{tb}_{ttg}_{tt}")
                        pvs.append(pv)
                    for d in range(16):
                        wv = wpool.tile([128, GL * hD], F32R, tag="wv", name=f"wv{tb}_{ttg}_{d}")
                        nc.sync.dma_start(wv[:], wvT[128 * d:128 * (d + 1), :])
                        for tt in range(2):
                            tloc = 2 * ttg + tt
                            nc.tensor.matmul(pvs[tt][:], xts[d][:, 128 * tloc:128 * (tloc + 1)],
                                             wv[:], start=(d == 0), stop=(d == 15))
                    for tt in range(2):
                        nc.vector.tensor_copy(VT[:][:, 4 * tb + 2 * ttg + tt, :], pvs[tt][:])
                # Q projection -> QT[:, m, tsl], m-groups of 2
                for mg in range(HL // 2):
                    pqs_ = []
                    for mm in range(2):
                        pq = psA.tile([128, 512], F32, tag=f"pq{mm}", bufs=1,
                                      name=f"pq{tb}_{mg}_{mm}")
                        pqs_.append(pq)
                    for d in range(16):
                        wq = wpool.tile([128, 256], F32R, tag="wq", name=f"wq{tb}_{mg}_{d}")
                        nc.sync.dma_start(wq[:], wqT[128 * d:128 * (d + 1), 256 * mg:256 * (mg + 1)])
                        for mm in range(2):
                            nc.tensor.matmul(pqs_[mm][:], wq[:, 128 * mm:128 * (mm + 1)],
                                             xts[d][:], start=(d == 0), stop=(d == 15))
                    qblks = []
                    for mm in range(2):
                        qb = qblkp.tile([128, 512], F32R, tag="qblk", name=f"qb{tb}_{mg}_{mm}")
                        nc.vector.tensor_copy(qb[:], pqs_[mm][:])
                        qblks.append(qb)
                    # Qsw projection for this head pair + rope + spill to DRAM
                    pqsw = psA.tile([128, 512], F32, tag="pqsw", bufs=1, name=f"pqsw{tb}_{mg}")
                    for d in range(16):
                        wqsw = wpool.tile([128, 128], F32R, tag="wqsw", name=f"wqsw{tb}_{mg}_{d}")
                        nc.sync.dma_start(wqsw[:], wqswT[128 * d:128 * (d + 1), 128 * mg:128 * (mg + 1)])
                        nc.tensor.matmul(pqsw[:], wqsw[:], xts[d][:], start=(d == 0), stop=(d == 15))
                    qsw = swpool.tile([128, 512], F32R, tag="qsw", name=f"qsw{tb}_{mg}")
                    nc.vector.tensor_copy(qsw[:], pqsw[:])
                    for hh in range(2):
                        h = 2 * mg + hh
                        b0 = ROPE * hh
                        qb = qblks[hh]
                        ts_ = rtmp.tile([ROPE, 512], F32R, tag="ts_", name=f"qts{tb}_{mg}_{hh}")
                        tc_ = rtmp.tile([ROPE, 512], F32R, tag="tc_", name=f"qtc{tb}_{mg}_{hh}")
                        nc.vector.tensor_mul(ts_[:], qsw[b0:b0 + ROPE, :], tsin[b0:b0 + ROPE, tsl])
                        nc.vector.tensor_mul(tc_[:], qb[0:ROPE, :], tcos[:, tsl])
                        nc.vector.tensor_tensor(out=qb[0:ROPE, :], in0=tc_[:], in1=ts_[:], op=AOp.add)
                        nc.sync.dma_start(QTd[:][:, h, tsl], qb[:])

        # ---------------- phase B: attention ----------------
        with ExitStack() as bctx:
            ptp = bctx.enter_context(tc.tile_pool(name="ptp", bufs=5))
            qhp = bctx.enter_context(tc.tile_pool(name="qhp", bufs=2))
            nrm = bctx.enter_context(tc.tile_pool(name="nrm", bufs=2))
            psB = bctx.enter_context(tc.tile_pool(name="psB", bufs=1, space="PSUM"))
            for h in range(HL):
                g = h // (HL // GL)
                qh = qhp.tile([128, T], F32R, tag="qh", name=f"qh{h}")
                nc.sync.dma_start(qh[:], QTd[:][:, h, :])
                for ib in range(NB):
                    isl = slice(512 * ib, 512 * (ib + 1))
                    qsl = qh[:, isl]
                    nj = 4 * ib + 4
                    po = psB.tile([128, 512], F32, tag="po", bufs=2, name=f"po{h}_{ib}")
                    pd = psB.tile([1, 512], F32, tag="pd", bufs=2, name=f"pd{h}_{ib}")
                    for j in range(nj):
                        st = psB.tile([128, 512], F32, tag="st", bufs=3, name=f"st{h}_{ib}_{j}")
                        diag = j >= 4 * ib
                        nc.tensor.matmul(st[:], KT[:][:, g, 128 * j:128 * (j + 1)], qsl,
                                         start=True, stop=not diag)
                        if diag:
                            nc.tensor.matmul(st[:], tid[:], tmask[:][:, j - 4 * ib, :],
                                             start=False, stop=True)
                        pt = ptp.tile([128, 512], F32R, tag="pt", name=f"pt{h}_{ib}_{j}")
                        nc.scalar.activation(pt[:], st[:], AF.Exp)
                        nc.tensor.matmul(po[:], VT[:][:, j, 128 * g:128 * (g + 1)], pt[:],
                                         start=(j == 0), stop=(j == nj - 1))
                        nc.tensor.matmul(pd[:], toc[:], pt[:],
                                         start=(j == 0), stop=(j == nj - 1))
                    dsb = nrm.tile([1, 512], F32, tag="dsb", name=f"dsb{h}_{ib}")
                    nc.vector.tensor_copy(dsb[:], pd[:])
                    rsb = nrm.tile([1, 512], F32R, tag="rsb", name=f"rsb{h}_{ib}")
                    with nc.allow_low_precision(reason="fp32r storage is fp32-width"):
                        nc.vector.reciprocal(rsb[:], dsb[:])
                    pr = psB.tile([128, 512], F32, tag="pr", bufs=1, name=f"pr{h}_{ib}")
                    nc.tensor.matmul(pr[:], tor[:], rsb[:], start=True, stop=True)
                    rps = nrm.tile([128, 512], F32, tag="rps", name=f"rps{h}_{ib}")
                    nc.vector.tensor_copy(rps[:], pr[:])
                    nc.vector.tensor_tensor(out=OT[:][:, h, isl], in0=po[:], in1=rps[:], op=AOp.mult)

        # ---------------- phase C: output projection (transposed) ----------------
        with ExitStack() as cctx:
            wop = cctx.enter_context(tc.tile_pool(name="wop", bufs=1))
            stg = cctx.enter_context(tc.tile_pool(name="stg", bufs=2))
            psC = cctx.enter_context(tc.tile_pool(name="psC", bufs=8, space="PSUM"))
            wo_t = []
            for j in range(HL):
                w = wop.tile([128, D], F32R, tag=f"wo{j}", name=f"wo{j}")
                nc.sync.dma_start(w[:], woT[128 * j:128 * (j + 1), :])
                wo_t.append(w)
            for m2 in range(16):
                msl = slice(128 * m2, 128 * (m2 + 1))
                stile = stg.tile([128, T], F32, tag="stile", name=f"stile{m2}")
                for tbl in range(NB):
                    pc = psC.tile([128, 512], F32, tag="pc", name=f"pc{m2}_{tbl}")
                    for j in range(HL):
                        nc.tensor.matmul(pc[:], wo_t[j][:, msl],
                                         OT[:][:, j, 512 * tbl:512 * (tbl + 1)],
                                         start=(j == 0), stop=(j == HL - 1))
                    nc.vector.tensor_copy(stile[:, 512 * tbl:512 * (tbl + 1)], pc[:])
                nc.sync.dma_start(outT[msl, :], stile[:])
    nc.compile()
    return nc


def prepare_inputs(x, cos, sin, Wq, Wkv, Wo, q_gain):
    """Host-side sharding + layout prep. Returns list of 8 in_maps."""
    x = np.asarray(x, np.float32)
    cos = np.asarray(cos, np.float32)
    sin = np.asarray(sin, np.float32)
    Wq = np.asarray(Wq, np.float32)
    Wkv = np.asarray(Wkv, np.float32)
    Wo = np.asarray(Wo, np.float32)
    q_gain = np.asarray(q_gain, np.float32)

    # rope tables: C64[c] = cos[:, c//2]; S64[2p] = -sin[p], S64[2p+1] = +sin[p]
    C64 = np.repeat(cos.T, 2, axis=0)              # [64, T]
    S64 = np.repeat(sin.T, 2, axis=0)
    S64[0::2] *= -1.0
    sinb = np.concatenate([S64, S64], axis=0)      # [128, T] (base-0 and base-64 slices)
    cosb = np.ascontiguousarray(C64)

    # additive causal masks for diagonal s-tiles, r = j - 4*ib
    p = np.arange(128)[:, None]
    f = np.arange(512)[None, :]
    maskb = np.zeros((128, 4, 512), np.float32)
    for r in range(4):
        maskb[:, r, :] = np.where(p + 128 * r > f, -1e9, 0.0)

    ident = np.eye(128, dtype=np.float32)
    ones_c = np.ones((128, 1), np.float32)
    ones_r = np.ones((1, 128), np.float32)

    part = (np.arange(ROPE) ^ 1)                   # partner channel within rope dims
    scale = 1.0 / np.sqrt(hD)

    xT = [np.ascontiguousarray(x[b].T) for b in range(B)]

    in_maps = []
    for c in range(8):
        b, hf = divmod(c, 2)
        heads = np.arange(hf * HL, (hf + 1) * HL)
        # Wq rows for this half, gain/scale folded
        Wq_h = Wq.reshape(H, hD, D)[heads] * (q_gain[heads, None, None] * scale)  # [HL, 128, D]
        Wq_sw = Wq_h[:, part, :]                                                  # [HL, 64, D]
        kvh = np.arange(hf * GL, (hf + 1) * GL)
        Wkv_r = Wkv.reshape(Hkv, 2 * hD, D)[kvh]                                  # [GL, 256, D]
        Wk_h = Wkv_r[:, :hD, :]
        Wv_h = Wkv_r[:, hD:, :]
        Wk_sw = Wk_h[:, part, :]
        Wo_h = Wo[:, hf * HL * hD:(hf + 1) * HL * hD]                             # [D, 1024]

        in_maps.append({
            "xT": xT[b],
            "wqT": np.ascontiguousarray(Wq_h.reshape(HL * hD, D).T),
            "wqswT": np.ascontiguousarray(Wq_sw.reshape(HL * ROPE, D).T),
            "wkT": np.ascontiguousarray(Wk_h.reshape(GL * hD, D).T),
            "wkswT": np.ascontiguousarray(Wk_sw.reshape(GL * ROPE, D).T),
            "wvT": np.ascontiguousarray(Wv_h.reshape(GL * hD, D).T),
            "woT": np.ascontiguousarray(Wo_h.T),
            "cosb": cosb, "sinb": sinb, "maskb": maskb,
            "ident": ident, "ones_c": ones_c, "ones_r": ones_r,
        })
    return in_maps


_NC_CACHE = {}


def kernel(x, cos, sin, Wq, Wkv, Wo, q_gain, _trace=False):
    if "nc" not in _NC_CACHE:
        _NC_CACHE["nc"] = build_nc()
    nc = _NC_CACHE["nc"]
    in_maps = prepare_inputs(x, cos, sin, Wq, Wkv, Wo, q_gain)
    res = run_bass_kernel_spmd(nc, in_maps, core_ids=list(range(8)), trace=_trace)
    if _trace:
        _NC_CACHE["last_results"] = res
    out = np.empty((B, T, D), np.float32)
    for b in range(B):
        acc = res.results[2 * b]["outT"] + res.results[2 * b + 1]["outT"]
        out[b] = acc.T
    return out


# revision 5
# speedup vs baseline: 1.0077x; 1.0077x over previous
"""Causal self-attention (GQA + partial RoPE + q_gain) Trainium2 Bass kernel.

Model: B=4, T=2048, D=2048, H=16 q-heads, Hkv=4 kv-heads, hD=128, ROPE=64.
Sharding: 8 cores = 4 batches x 2 head-halves (heads h*8..h*8+7, kv heads 2h, 2h+1).
Wq/Wkv column-sharded, Wo row-sharded; host sums the two partial outputs per batch.

Per-core dataflow (all matmuls fp32r = full PE rate, fp32 storage):
  phase A: projections. Q.T/K.T in [channel, token] layout (W stationary, x.T moving),
           V in natural [token, channel] layout (x.T stationary, Wv.T moving).
           Partial-RoPE via "swap projection": an extra projection of the partner
           rope channels makes RoPE 3 partition-aligned elementwise ops on DVE
           (cross-partition combines are illegal on DVE).
           q_gain and 1/sqrt(hD) are folded into Wq host-side (rotation commutes
           with per-head scalars).
  phase B: attention per (head, 512-wide i-block). Scores computed TRANSPOSED
           S.T[s, i] (K.T stationary, Q.T moving) so that P.T = exp(S.T) feeds the
           AV matmul directly (V natural stationary, P.T moving) -> O.T[c, i].
           Causal masking: additive -1e9 masks accumulated into the scores PSUM via
           an identity-stationary matmul (diagonal tiles only); exp of masked = 0.
           Softmax denominators D[i] = colsums of P.T via ones-stationary (M=1)
           matmuls accumulated alongside AV; 1/D replicated to 128 partitions via a
           K=1 ones matmul; O.T normalized during the PSUM->SBUF copy.
  phase C: output projection, TRANSPOSED: out.T[dout, t] (Wo.T-tile stationary,
           O.T moving). Host adds the two partials per batch and transposes back.
"""
import numpy as np

import concourse.bass as bass
import concourse.tile as tile
from concourse import bacc, mybir
from concourse.bass_utils import run_bass_kernel_spmd
from contextlib import ExitStack

F32 = mybir.dt.float32
F32R = mybir.dt.float32r
AF = mybir.ActivationFunctionType
AOp = mybir.AluOpType

B, T, D = 4, 2048, 2048
H, Hkv = 16, 4
hD = 128
ROPE = 64
NB = T // 512          # 4 blocks of 512 tokens
HL = H // 2            # 8 heads per core
GL = Hkv // 2          # 2 kv heads per core


def build_nc():
    nc = bacc.Bacc(trn_type="TRN2", target_bir_lowering=False, debug=False)
    xT = nc.dram_tensor("xT", [D, T], F32R, kind="ExternalInput").ap()
    wqT = nc.dram_tensor("wqT", [D, HL * hD], F32R, kind="ExternalInput").ap()
    wqswT = nc.dram_tensor("wqswT", [D, HL * ROPE], F32R, kind="ExternalInput").ap()
    wkT = nc.dram_tensor("wkT", [D, GL * hD], F32R, kind="ExternalInput").ap()
    wkswT = nc.dram_tensor("wkswT", [D, GL * ROPE], F32R, kind="ExternalInput").ap()
    wvT = nc.dram_tensor("wvT", [D, GL * hD], F32R, kind="ExternalInput").ap()
    woT = nc.dram_tensor("woT", [HL * hD, D], F32R, kind="ExternalInput").ap()
    cosb = nc.dram_tensor("cosb", [ROPE, T], F32R, kind="ExternalInput").ap()
    sinb = nc.dram_tensor("sinb", [2 * ROPE, T], F32R, kind="ExternalInput").ap()
    maskb = nc.dram_tensor("maskb", [128, 4, 512], F32R, kind="ExternalInput").ap()
    ident = nc.dram_tensor("ident", [128, 128], F32R, kind="ExternalInput").ap()
    ones_c = nc.dram_tensor("ones_c", [128, 1], F32R, kind="ExternalInput").ap()
    ones_r = nc.dram_tensor("ones_r", [1, 128], F32R, kind="ExternalInput").ap()
    outT = nc.dram_tensor("outT", [D, T], F32, kind="ExternalOutput").ap()

    with tile.TileContext(nc) as tc, ExitStack() as ctx:
        const = ctx.enter_context(tc.tile_pool(name="const", bufs=1))
        persist = ctx.enter_context(tc.tile_pool(name="persist", bufs=1))

        tid = const.tile([128, 128], F32R, tag="tid")
        nc.sync.dma_start(tid[:], ident)
        toc = const.tile([128, 1], F32R, tag="toc")
        nc.sync.dma_start(toc[:], ones_c)
        tor = const.tile([1, 128], F32R, tag="tor")
        nc.sync.dma_start(tor[:], ones_r)
        tmask = const.tile([128, 4, 512], F32R, tag="tmask")
        nc.sync.dma_start(tmask[:], maskb)

        dramp = ctx.enter_context(tc.tile_pool(name="dramp", bufs=1, space="DRAM"))
        QTd = dramp.tile([128, HL, T], F32R, tag="QTd")
        KT = persist.tile([128, GL, T], F32R, tag="KT")
        VT = persist.tile([128, T // 128, GL * hD], F32R, tag="VT")
        OT = persist.tile([128, HL, T], F32R, tag="OT")

        # ---------------- phase A: projections + rope ----------------
        with ExitStack() as actx:
            tabs = actx.enter_context(tc.tile_pool(name="tabs", bufs=1))
            xpool = actx.enter_context(tc.tile_pool(name="xp", bufs=16))
            wpool = actx.enter_context(tc.tile_pool(name="wp", bufs=3))
            swpool = actx.enter_context(tc.tile_pool(name="swp", bufs=2))
            rtmp = actx.enter_context(tc.tile_pool(name="rtmp", bufs=2))
            qblkp = actx.enter_context(tc.tile_pool(name="qblkp", bufs=3))
            psA = actx.enter_context(tc.tile_pool(name="psA", bufs=1, space="PSUM"))

            tcos = tabs.tile([ROPE, T], F32R, tag="tcos")
            nc.sync.dma_start(tcos[:], cosb)
            tsin = tabs.tile([2 * ROPE, T], F32R, tag="tsin")
            nc.sync.dma_start(tsin[:], sinb)

            for tb in range(NB):
              with nc.named_scope(f"A{tb}"):
                tsl = slice(512 * tb, 512 * (tb + 1))
                xts = []
                for d in range(16):
                    xt = xpool.tile([128, 512], F32R, tag="xt", name=f"xt{tb}_{d}")
                    nc.sync.dma_start(xt[:], xT[128 * d:128 * (d + 1), tsl])
                    xts.append(xt)

                # K projection -> KT[:, g, tsl]  (weights streamed per use)
                for g in range(GL):
                    pk = psA.tile([128, 512], F32, tag="pk", bufs=2, name=f"pk{tb}_{g}")
                    for d in range(16):
                        wk = wpool.tile([128, 128], F32R, tag="wk", name=f"wk{tb}_{g}_{d}")
                        nc.sync.dma_start(wk[:], wkT[128 * d:128 * (d + 1), 128 * g:128 * (g + 1)])
                        nc.tensor.matmul(pk[:], wk[:], xts[d][:], start=(d == 0), stop=(d == 15))
                    nc.vector.tensor_copy(KT[:][:, g, tsl], pk[:])
                # Ksw projection + K rope
                pksw = psA.tile([128, 512], F32, tag="pksw", bufs=1, name=f"pksw{tb}")
                for d in range(16):
                    wksw = wpool.tile([128, GL * ROPE], F32R, tag="wksw", name=f"wksw{tb}_{d}")
                    nc.sync.dma_start(wksw[:], wkswT[128 * d:128 * (d + 1), :])
                    nc.tensor.matmul(pksw[:], wksw[:], xts[d][:], start=(d == 0), stop=(d == 15))
                ksw = swpool.tile([128, 512], F32R, tag="ksw", name=f"ksw{tb}")
                nc.vector.tensor_copy(ksw[:], pksw[:])
                for g in range(GL):
                    b0 = ROPE * g
                    ts_ = rtmp.tile([ROPE, 512], F32R, tag="ts_", name=f"kts{tb}_{g}")
                    tc_ = rtmp.tile([ROPE, 512], F32R, tag="tc_", name=f"ktc{tb}_{g}")
                    nc.vector.tensor_mul(ts_[:], ksw[b0:b0 + ROPE, :], tsin[b0:b0 + ROPE, tsl])
                    nc.vector.tensor_mul(tc_[:], KT[:][0:ROPE, g, tsl], tcos[:, tsl])
                    nc.vector.tensor_tensor(out=KT[:][0:ROPE, g, tsl], in0=tc_[:], in1=ts_[:], op=AOp.add)
                # V projection (natural layout): VT[:, 4*tb+tt, :], tt-groups of 2
                for ttg in range(2):
                    pvs = []
                    for tt in range(2):
                        pv = psA.tile([128, GL * hD], F32, tag=f"pv{tt}", bufs=1,
                                      name=f"pv{tb}_{ttg}_{tt}")
                        pvs.append(pv)
                    for d in range(16):
                        wv = wpool.tile([128, GL * hD], F32R, tag="wv", name=f"wv{tb}_{ttg}_{d}")
                        nc.sync.dma_start(wv[:], wvT[128 * d:128 * (d + 1), :])
                        for tt in range(2):
                            tloc = 2 * ttg + tt
                            nc.tensor.matmul(pvs[tt][:], xts[d][:, 128 * tloc:128 * (tloc + 1)],
                                             wv[:], start=(d == 0), stop=(d == 15))
                    for tt in range(2):
                        nc.vector.tensor_copy(VT[:][:, 4 * tb + 2 * ttg + tt, :], pvs[tt][:])
                # Q projection -> QT[:, m, tsl], m-groups of 2
                for mg in range(HL // 2):
                    pqs_ = []
                    for mm in range(2):
                        pq = psA.tile([128, 512], F32, tag=f"pq{mm}", bufs=1,
                                      name=f"pq{tb}_{mg}_{mm}")
                        pqs_.append(pq)
                    for d in range(16):
                        wq = wpool.tile([128, 256], F32R, tag="wq", name=f"wq{tb}_{mg}_{d}")
                        nc.sync.dma_start(wq[:], wqT[128 * d:128 * (d + 1), 256 * mg:256 * (mg + 1)])
                        for mm in range(2):
                            nc.tensor.matmul(pqs_[mm][:], wq[:, 128 * mm:128 * (mm + 1)],
                                             xts[d][:], start=(d == 0), stop=(d == 15))
                    qblks = []
                    for mm in range(2):
                        qb = qblkp.tile([128, 512], F32R, tag="qblk", name=f"qb{tb}_{mg}_{mm}")
                        nc.vector.tensor_copy(qb[:], pqs_[mm][:])
                        qblks.append(qb)
                    # Qsw projection for this head pair + rope + spill to DRAM
                    pqsw = psA.tile([128, 512], F32, tag="pqsw", bufs=1, name=f"pqsw{tb}_{mg}")
                    for d in range(16):
                        wqsw = wpool.tile([128, 128], F32R, tag="wqsw", name=f"wqsw{tb}_{mg}_{d}")
                        nc.sync.dma_start(wqsw[:], wqswT[128 * d:128 * (d + 1), 128 * mg:128 * (mg + 1)])
                        nc.tensor.matmul(pqsw[:], wqsw[:], xts[d][:], start=(d == 0), stop=(d == 15))
                    qsw = swpool.tile([128, 512], F32R, tag="qsw", name=f"qsw{tb}_{mg}")
                    nc.vector.tensor_copy(qsw[:], pqsw[:])
                    for hh in range(2):
                        h = 2 * mg + hh
                        b0 = ROPE * hh
                        qb = qblks[hh]
                        ts_ = rtmp.tile([ROPE, 512], F32R, tag="ts_", name=f"qts{tb}_{mg}_{hh}")
                        tc_ = rtmp.tile([ROPE, 512], F32R, tag="tc_", name=f"qtc{tb}_{mg}_{hh}")
                        nc.vector.tensor_mul(ts_[:], qsw[b0:b0 + ROPE, :], tsin[b0:b0 + ROPE, tsl])
                        nc.vector.tensor_mul(tc_[:], qb[0:ROPE, :], tcos[:, tsl])
                        nc.vector.tensor_tensor(out=qb[0:ROPE, :], in0=tc_[:], in1=ts_[:], op=AOp.add)
                        nc.sync.dma_start(QTd[:][:, h, tsl], qb[:])

        # ---------------- phase B: attention ----------------
        with ExitStack() as bctx:
            ptp = bctx.enter_context(tc.tile_pool(name="ptp", bufs=5))
            qhp = bctx.enter_context(tc.tile_pool(name="qhp", bufs=2))
            nrm = bctx.enter_context(tc.tile_pool(name="nrm", bufs=2))
            psB = bctx.enter_context(tc.tile_pool(name="psB", bufs=1, space="PSUM"))
            for h in range(HL):
              with nc.named_scope(f"B{h}"):
                g = h // (HL // GL)
                qh = qhp.tile([128, T], F32R, tag="qh", name=f"qh{h}")
                nc.sync.dma_start(qh[:], QTd[:][:, h, :])
                for ib in range(NB):
                    isl = slice(512 * ib, 512 * (ib + 1))
                    qsl = qh[:, isl]
                    nj = 4 * ib + 4
                    po = psB.tile([128, 512], F32, tag="po", bufs=2, name=f"po{h}_{ib}")
                    pd = psB.tile([1, 512], F32, tag="pd", bufs=2, name=f"pd{h}_{ib}")
                    for j in range(nj):
                        st = psB.tile([128, 512], F32, tag="st", bufs=3, name=f"st{h}_{ib}_{j}")
                        diag = j >= 4 * ib
                        nc.tensor.matmul(st[:], KT[:][:, g, 128 * j:128 * (j + 1)], qsl,
                                         start=True, stop=not diag)
                        if diag:
                            nc.tensor.matmul(st[:], tid[:], tmask[:][:, j - 4 * ib, :],
                                             start=False, stop=True)
                        pt = ptp.tile([128, 512], F32R, tag="pt", name=f"pt{h}_{ib}_{j}")
                        nc.scalar.activation(pt[:], st[:], AF.Exp)
                        nc.tensor.matmul(po[:], VT[:][:, j, 128 * g:128 * (g + 1)], pt[:],
                                         start=(j == 0), stop=(j == nj - 1))
                        nc.tensor.matmul(pd[:], toc[:], pt[:],
                                         start=(j == 0), stop=(j == nj - 1))
                    dsb = nrm.tile([1, 512], F32, tag="dsb", name=f"dsb{h}_{ib}")
                    nc.vector.tensor_copy(dsb[:], pd[:])
                    rsb = nrm.tile([1, 512], F32R, tag="rsb", name=f"rsb{h}_{ib}")
                    with nc.allow_low_precision(reason="fp32r storage is fp32-width"):
                        nc.vector.reciprocal(rsb[:], dsb[:])
                    pr = psB.tile([128, 512], F32, tag="pr", bufs=1, name=f"pr{h}_{ib}")
                    nc.tensor.matmul(pr[:], tor[:], rsb[:], start=True, stop=True)
                    rps = nrm.tile([128, 512], F32, tag="rps", name=f"rps{h}_{ib}")
                    nc.vector.tensor_copy(rps[:], pr[:])
                    nc.vector.tensor_tensor(out=OT[:][:, h, isl], in0=po[:], in1=rps[:], op=AOp.mult)

        # ---------------- phase C: output projection (transposed) ----------------
        with ExitStack() as cctx:
            wop = cctx.enter_context(tc.tile_pool(name="wop", bufs=1))
            stg = cctx.enter_context(tc.tile_pool(name="stg", bufs=2))
            psC = cctx.enter_context(tc.tile_pool(name="psC", bufs=8, space="PSUM"))
            wo_t = []
            for j in range(HL):
                w = wop.tile([128, D], F32R, tag=f"wo{j}", name=f"wo{j}")
                nc.sync.dma_start(w[:], woT[128 * j:128 * (j + 1), :])
                wo_t.append(w)
            for m2 in range(16):
              with nc.named_scope(f"C{m2 // 4}"):
                msl = slice(128 * m2, 128 * (m2 + 1))
                stile = stg.tile([128, T], F32, tag="stile", name=f"stile{m2}")
                for tbl in range(NB):
                    pc = psC.tile([128, 512], F32, tag="pc", name=f"pc{m2}_{tbl}")
                    for j in range(HL):
                        nc.tensor.matmul(pc[:], wo_t[j][:, msl],
                                         OT[:][:, j, 512 * tbl:512 * (tbl + 1)],
                                         start=(j == 0), stop=(j == HL - 1))
                    nc.vector.tensor_copy(stile[:, 512 * tbl:512 * (tbl + 1)], pc[:])
                nc.sync.dma_start(outT[msl, :], stile[:])
    nc.compile()
    return nc


def prepare_inputs(x, cos, sin, Wq, Wkv, Wo, q_gain):
    """Host-side sharding + layout prep. Returns list of 8 in_maps."""
    x = np.asarray(x, np.float32)
    cos = np.asarray(cos, np.float32)
    sin = np.asarray(sin, np.float32)
    Wq = np.asarray(Wq, np.float32)
    Wkv = np.asarray(Wkv, np.float32)
    Wo = np.asarray(Wo, np.float32)
    q_gain = np.asarray(q_gain, np.float32)

    # rope tables: C64[c] = cos[:, c//2]; S64[2p] = -sin[p], S64[2p+1] = +sin[p]
    C64 = np.repeat(cos.T, 2, axis=0)              # [64, T]
    S64 = np.repeat(sin.T, 2, axis=0)
    S64[0::2] *= -1.0
    sinb = np.concatenate([S64, S64], axis=0)      # [128, T] (base-0 and base-64 slices)
    cosb = np.ascontiguousarray(C64)

    # additive causal masks for diagonal s-tiles, r = j - 4*ib
    p = np.arange(128)[:, None]
    f = np.arange(512)[None, :]
    maskb = np.zeros((128, 4, 512), np.float32)
    for r in range(4):
        maskb[:, r, :] = np.where(p + 128 * r > f, -1e9, 0.0)

    ident = np.eye(128, dtype=np.float32)
    ones_c = np.ones((128, 1), np.float32)
    ones_r = np.ones((1, 128), np.float32)

    part = (np.arange(ROPE) ^ 1)                   # partner channel within rope dims
    scale = 1.0 / np.sqrt(hD)

    xT = [np.ascontiguousarray(x[b].T) for b in range(B)]

    in_maps = []
    for c in range(8):
        b, hf = divmod(c, 2)
        heads = np.arange(hf * HL, (hf + 1) * HL)
        # Wq rows for this half, gain/scale folded
        Wq_h = Wq.reshape(H, hD, D)[heads] * (q_gain[heads, None, None] * scale)  # [HL, 128, D]
        Wq_sw = Wq_h[:, part, :]                                                  # [HL, 64, D]
        kvh = np.arange(hf * GL, (hf + 1) * GL)
        Wkv_r = Wkv.reshape(Hkv, 2 * hD, D)[kvh]                                  # [GL, 256, D]
        Wk_h = Wkv_r[:, :hD, :]
        Wv_h = Wkv_r[:, hD:, :]
        Wk_sw = Wk_h[:, part, :]
        Wo_h = Wo[:, hf * HL * hD:(hf + 1) * HL * hD]                             # [D, 1024]

        in_maps.append({
            "xT": xT[b],
            "wqT": np.ascontiguousarray(Wq_h.reshape(HL * hD, D).T),
            "wqswT": np.ascontiguousarray(Wq_sw.reshape(HL * ROPE, D).T),
            "wkT": np.ascontiguousarray(Wk_h.reshape(GL * hD, D).T),
            "wkswT": np.ascontiguousarray(Wk_sw.reshape(GL * ROPE, D).T),
            "wvT": np.ascontiguousarray(Wv_h.reshape(GL * hD, D).T),
            "woT": np.ascontiguousarray(Wo_h.T),
            "cosb": cosb, "sinb": sinb, "maskb": maskb,
            "ident": ident, "ones_c": ones_c, "ones_r": ones_r,
        })
    return in_maps


_NC_CACHE = {}


def kernel(x, cos, sin, Wq, Wkv, Wo, q_gain, _trace=False):
    if "nc" not in _NC_CACHE:
        _NC_CACHE["nc"] = build_nc()
    nc = _NC_CACHE["nc"]
    in_maps = prepare_inputs(x, cos, sin, Wq, Wkv, Wo, q_gain)
    res = run_bass_kernel_spmd(nc, in_maps, core_ids=list(range(8)), trace=_trace)
    if _trace:
        _NC_CACHE["last_results"] = res
    out = np.empty((B, T, D), np.float32)
    for b in range(B):
        acc = res.results[2 * b]["outT"] + res.results[2 * b + 1]["outT"]
        out[b] = acc.T
    return out


# revision 7
# speedup vs baseline: 1.6730x; 1.6602x over previous
"""Causal self-attention (GQA + partial RoPE + q_gain) Trainium2 Bass kernel.

Model: B=4, T=2048, D=2048, H=16 q-heads, Hkv=4 kv-heads, hD=128, ROPE=64.
Sharding: 8 cores = 4 batches x 2 head-halves (heads hf*8..hf*8+7, kv heads 2hf, 2hf+1).
Wq/Wkv column-sharded, Wo row-sharded; host sums the two partial outputs per batch.

All matmuls fp32r (full PE rate, fp32 storage, tf32-like precision ~1.5e-4).

Per-core dataflow:
  phase A: projections. Q.T/K.T in [channel, token] layout (W stationary, x.T moving),
           V natural [token, channel] (x.T stationary, Wv.T moving). Channels of the
           rope half are de-interleaved host-side (x1 -> 0:32, x2 -> 32:64, consistent
           for Q and K so scores are invariant). RoPE: partner half obtained by two
           32-partition SBUF->SBUF swap DMAs (gpsimd), then 3 aligned DVE ops:
           rope_out[0:64] = qb[0:64]*C + swap*S  with C=[cos;cos], S=[-sin;+sin].
           q_gain and 1/sqrt(hD) folded into Wq host-side. Q.T spilled to DRAM.
  phase B: attention per (head, 512 i-block). Scores TRANSPOSED S.T[s,i] (K.T
           stationary, Q.T moving) so P.T = exp(S.T) feeds AV directly (V natural
           stationary, P.T moving) -> O.T[c,i]. Causal mask: additive -1e9 tiles
           accumulated into scores PSUM via identity-stationary matmul (diagonal
           tiles). Denominators D = ones-stationary (M=1) matmuls on P.T; 1/D via
           fast-approx reciprocal, replicated across partitions by a K=1 matmul;
           O.T normalized during PSUM->SBUF copy.
  phase C: output projection TRANSPOSED: out.T[dout,t] (Wo.T-tile stationary, O.T
           moving). Host adds the two per-batch partials and transposes back.
"""
import numpy as np

import concourse.bass as bass
import concourse.tile as tile
from concourse import bacc, mybir
from concourse.bass_utils import run_bass_kernel_spmd
from contextlib import ExitStack

F32 = mybir.dt.float32
F32R = mybir.dt.float32r
AF = mybir.ActivationFunctionType
AOp = mybir.AluOpType

B, T, D = 4, 2048, 2048
H, Hkv = 16, 4
hD = 128
ROPE = 64
NB = T // 512          # 4 blocks of 512 tokens
HL = H // 2            # 8 heads per core
GL = Hkv // 2          # 2 kv heads per core


def build_nc():
    nc = bacc.Bacc(trn_type="TRN2", target_bir_lowering=False, debug=False)
    xT = nc.dram_tensor("xT", [D, T], F32R, kind="ExternalInput").ap()
    wqT = nc.dram_tensor("wqT", [D, HL * hD], F32R, kind="ExternalInput").ap()
    wkT = nc.dram_tensor("wkT", [D, GL * hD], F32R, kind="ExternalInput").ap()
    wvT = nc.dram_tensor("wvT", [D, GL * hD], F32R, kind="ExternalInput").ap()
    woT = nc.dram_tensor("woT", [HL * hD, D], F32R, kind="ExternalInput").ap()
    cosb = nc.dram_tensor("cosb", [ROPE, T], F32R, kind="ExternalInput").ap()
    sinb = nc.dram_tensor("sinb", [ROPE, T], F32R, kind="ExternalInput").ap()
    maskb = nc.dram_tensor("maskb", [128, 4, 512], F32R, kind="ExternalInput").ap()
    ident = nc.dram_tensor("ident", [128, 128], F32R, kind="ExternalInput").ap()
    ones_c = nc.dram_tensor("ones_c", [128, 1], F32R, kind="ExternalInput").ap()
    ones_r = nc.dram_tensor("ones_r", [1, 128], F32R, kind="ExternalInput").ap()
    outT = nc.dram_tensor("outT", [D, T], F32, kind="ExternalOutput").ap()

    xTr = xT.rearrange("(n p) t -> p n t", p=128)      # [128, 16, 2048]
    wqTr = wqT.rearrange("(n p) m -> p n m", p=128)    # [128, 16, 1024]
    wkTr = wkT.rearrange("(n p) m -> p n m", p=128)    # [128, 16, 256]
    wvTr = wvT.rearrange("(n p) m -> p n m", p=128)

    with tile.TileContext(nc) as tc, ExitStack() as ctx:
        const = ctx.enter_context(tc.tile_pool(name="const", bufs=1))
        persist = ctx.enter_context(tc.tile_pool(name="persist", bufs=1))
        dramp = ctx.enter_context(tc.tile_pool(name="dramp", bufs=1, space="DRAM"))

        tid = const.tile([128, 128], F32R, tag="tid")
        nc.sync.dma_start(tid[:], ident)
        toc = const.tile([128, 1], F32R, tag="toc")
        nc.sync.dma_start(toc[:], ones_c)
        tor = const.tile([1, 128], F32R, tag="tor")
        nc.sync.dma_start(tor[:], ones_r)
        tmask = const.tile([128, 4, 512], F32R, tag="tmask")
        nc.sync.dma_start(tmask[:], maskb)

        QTd = dramp.tile([128, HL, T], F32R, tag="QTd")
        KT = persist.tile([128, GL, T], F32R, tag="KT")
        VT = persist.tile([128, T // 128, GL * hD], F32R, tag="VT")

        # ---------------- phase A: projections + rope ----------------
        with ExitStack() as actx:
            tabs = actx.enter_context(tc.tile_pool(name="tabs", bufs=1))
            xpool = actx.enter_context(tc.tile_pool(name="xp", bufs=17))
            wkvp = actx.enter_context(tc.tile_pool(name="wkvp", bufs=1))
            wqp = actx.enter_context(tc.tile_pool(name="wqp", bufs=2))
            swpool = actx.enter_context(tc.tile_pool(name="swp", bufs=2))
            rtmp = actx.enter_context(tc.tile_pool(name="rtmp", bufs=2))
            qblkp = actx.enter_context(tc.tile_pool(name="qblkp", bufs=3))
            psA = actx.enter_context(tc.tile_pool(name="psA", bufs=1, space="PSUM"))

            tcos = tabs.tile([ROPE, T], F32R, tag="tcos")
            nc.sync.dma_start(tcos[:], cosb)
            tsin = tabs.tile([ROPE, T], F32R, tag="tsin")
            nc.sync.dma_start(tsin[:], sinb)

            for tb in range(NB):
              with nc.named_scope(f"A{tb}"):
                tsl = slice(512 * tb, 512 * (tb + 1))
                xts = []
                for d in range(16):
                    xt = xpool.tile([128, 512], F32R, tag="xt", name=f"xt{tb}_{d}")
                    nc.sync.dma_start(xt[:], xTr[:, d, tsl])
                    xts.append(xt)
                # K+V weights: one DMA each per t-block
                wkg = wkvp.tile([128, 16, GL * hD], F32R, tag="wkg", bufs=1, name=f"wkg{tb}")
                nc.sync.dma_start(wkg[:], wkTr)
                wvg = wkvp.tile([128, 16, GL * hD], F32R, tag="wvg", bufs=1, name=f"wvg{tb}")
                nc.sync.dma_start(wvg[:], wvTr)

                # K projection -> KT[:, g, tsl], then rope via swap-DMA + 3 DVE ops
                for g in range(GL):
                    pk = psA.tile([128, 512], F32, tag="pk", bufs=2, name=f"pk{tb}_{g}")
                    for d in range(16):
                        nc.tensor.matmul(pk[:], wkg[:][:, d, 128 * g:128 * (g + 1)],
                                         xts[d][:], start=(d == 0), stop=(d == 15))
                    nc.vector.tensor_copy(KT[:][:, g, tsl], pk[:])
                    ksw = swpool.tile([ROPE, 512], F32R, tag="ksw", name=f"ksw{tb}_{g}")
                    nc.gpsimd.dma_start(ksw[0:32, :], KT[:][32:64, g, tsl])
                    nc.gpsimd.dma_start(ksw[32:64, :], KT[:][0:32, g, tsl])
                    ts_ = rtmp.tile([ROPE, 512], F32R, tag="ts_", name=f"kts{tb}_{g}")
                    tc_ = rtmp.tile([ROPE, 512], F32R, tag="tc_", name=f"ktc{tb}_{g}")
                    nc.vector.tensor_mul(ts_[:], ksw[:], tsin[:, tsl])
                    nc.vector.tensor_mul(tc_[:], KT[:][0:ROPE, g, tsl], tcos[:, tsl])
                    nc.vector.tensor_tensor(out=KT[:][0:ROPE, g, tsl], in0=tc_[:], in1=ts_[:], op=AOp.add)
                # V projection (natural layout): VT[:, 4*tb+tt, :], tt-groups of 2
                for ttg in range(2):
                    pvs = []
                    for tt in range(2):
                        pv = psA.tile([128, GL * hD], F32, tag=f"pv{tt}", bufs=1,
                                      name=f"pv{tb}_{ttg}_{tt}")
                        pvs.append(pv)
                    for d in range(16):
                        for tt in range(2):
                            tloc = 2 * ttg + tt
                            nc.tensor.matmul(pvs[tt][:], xts[d][:, 128 * tloc:128 * (tloc + 1)],
                                             wvg[:][:, d, :], start=(d == 0), stop=(d == 15))
                    for tt in range(2):
                        nc.vector.tensor_copy(VT[:][:, 4 * tb + 2 * ttg + tt, :], pvs[tt][:])
                # Q projection (m-groups of 2) + rope + spill to DRAM
                for mg in range(HL // 2):
                    wqg = wqp.tile([128, 16, 256], F32R, tag="wqg", name=f"wqg{tb}_{mg}")
                    nc.sync.dma_start(wqg[:], wqTr[:, :, 256 * mg:256 * (mg + 1)])
                    pqs_ = []
                    for mm in range(2):
                        pq = psA.tile([128, 512], F32, tag=f"pq{mm}", bufs=1,
                                      name=f"pq{tb}_{mg}_{mm}")
                        pqs_.append(pq)
                    for d in range(16):
                        for mm in range(2):
                            nc.tensor.matmul(pqs_[mm][:], wqg[:][:, d, 128 * mm:128 * (mm + 1)],
                                             xts[d][:], start=(d == 0), stop=(d == 15))
                    for mm in range(2):
                        h = 2 * mg + mm
                        qb = qblkp.tile([128, 512], F32R, tag="qblk", name=f"qb{tb}_{mg}_{mm}")
                        nc.vector.tensor_copy(qb[:], pqs_[mm][:])
                        qsw = swpool.tile([ROPE, 512], F32R, tag="qsw", name=f"qsw{tb}_{mg}_{mm}")
                        nc.gpsimd.dma_start(qsw[0:32, :], qb[32:64, :])
                        nc.gpsimd.dma_start(qsw[32:64, :], qb[0:32, :])
                        ts_ = rtmp.tile([ROPE, 512], F32R, tag="ts_", name=f"qts{tb}_{mg}_{mm}")
                        tc_ = rtmp.tile([ROPE, 512], F32R, tag="tc_", name=f"qtc{tb}_{mg}_{mm}")
                        nc.vector.tensor_mul(ts_[:], qsw[:], tsin[:, tsl])
                        nc.vector.tensor_mul(tc_[:], qb[0:ROPE, :], tcos[:, tsl])
                        nc.vector.tensor_tensor(out=qb[0:ROPE, :], in0=tc_[:], in1=ts_[:], op=AOp.add)
                        nc.sync.dma_start(QTd[:][:, h, tsl], qb[:])

        # OT pool opened after phase A space is released; lives through B + C
        otp_pool = ctx.enter_context(tc.tile_pool(name="otp_pool", bufs=1))
        OT = otp_pool.tile([128, HL, T], F32R, tag="OT")

        # ---------------- phase B: attention ----------------
        with ExitStack() as bctx:
            ptp = bctx.enter_context(tc.tile_pool(name="ptp", bufs=5))
            qhp = bctx.enter_context(tc.tile_pool(name="qhp", bufs=2))
            nrm = bctx.enter_context(tc.tile_pool(name="nrm", bufs=2))
            psB = bctx.enter_context(tc.tile_pool(name="psB", bufs=1, space="PSUM"))
            for h in range(HL):
              with nc.named_scope(f"B{h}"):
                g = h // (HL // GL)
                qh = qhp.tile([128, T], F32R, tag="qh", name=f"qh{h}")
                nc.sync.dma_start(qh[:], QTd[:][:, h, :])
                for ib in range(NB):
                    isl = slice(512 * ib, 512 * (ib + 1))
                    qsl = qh[:, isl]
                    nj = 4 * ib + 4
                    po = psB.tile([128, 512], F32, tag="po", bufs=2, name=f"po{h}_{ib}")
                    pd = psB.tile([1, 512], F32, tag="pd", bufs=2, name=f"pd{h}_{ib}")
                    for j in range(nj):
                        st = psB.tile([128, 512], F32, tag="st", bufs=3, name=f"st{h}_{ib}_{j}")
                        diag = j >= 4 * ib
                        nc.tensor.matmul(st[:], KT[:][:, g, 128 * j:128 * (j + 1)], qsl,
                                         start=True, stop=not diag)
                        if diag:
                            nc.tensor.matmul(st[:], tid[:], tmask[:][:, j - 4 * ib, :],
                                             start=False, stop=True)
                        pt = ptp.tile([128, 512], F32R, tag="pt", name=f"pt{h}_{ib}_{j}")
                        nc.scalar.activation(pt[:], st[:], AF.Exp)
                        nc.tensor.matmul(po[:], VT[:][:, j, 128 * g:128 * (g + 1)], pt[:],
                                         start=(j == 0), stop=(j == nj - 1))
                        nc.tensor.matmul(pd[:], toc[:], pt[:],
                                         start=(j == 0), stop=(j == nj - 1))
                    dsb = nrm.tile([1, 512], F32, tag="dsb", name=f"dsb{h}_{ib}")
                    nc.vector.tensor_copy(dsb[:], pd[:])
                    rsb = nrm.tile([1, 512], F32, tag="rsb", name=f"rsb{h}_{ib}")
                    nc.vector.reciprocal_approx_fast(rsb[:], dsb[:])
                    rsr = nrm.tile([1, 512], F32R, tag="rsr", name=f"rsr{h}_{ib}")
                    nc.vector.tensor_copy(rsr[:], rsb[:])
                    pr = psB.tile([128, 512], F32, tag="pr", bufs=1, name=f"pr{h}_{ib}")
                    nc.tensor.matmul(pr[:], tor[:], rsr[:], start=True, stop=True)
                    rps = nrm.tile([128, 512], F32, tag="rps", name=f"rps{h}_{ib}")
                    nc.vector.tensor_copy(rps[:], pr[:])
                    nc.vector.tensor_tensor(out=OT[:][:, h, isl], in0=po[:], in1=rps[:], op=AOp.mult)

        # ---------------- phase C: output projection (transposed) ----------------
        with ExitStack() as cctx:
            wop = cctx.enter_context(tc.tile_pool(name="wop", bufs=1))
            stg = cctx.enter_context(tc.tile_pool(name="stg", bufs=2))
            psC = cctx.enter_context(tc.tile_pool(name="psC", bufs=8, space="PSUM"))
            wo_t = []
            for j in range(HL):
                w = wop.tile([128, D], F32R, tag=f"wo{j}", name=f"wo{j}")
                nc.sync.dma_start(w[:], woT[128 * j:128 * (j + 1), :])
                wo_t.append(w)
            for m2 in range(16):
              with nc.named_scope(f"C{m2 // 4}"):
                msl = slice(128 * m2, 128 * (m2 + 1))
                stile = stg.tile([128, T], F32, tag="stile", name=f"stile{m2}")
                for tbl in range(NB):
                    pc = psC.tile([128, 512], F32, tag="pc", name=f"pc{m2}_{tbl}")
                    for j in range(HL):
                        nc.tensor.matmul(pc[:], wo_t[j][:, msl],
                                         OT[:][:, j, 512 * tbl:512 * (tbl + 1)],
                                         start=(j == 0), stop=(j == HL - 1))
                    nc.vector.tensor_copy(stile[:, 512 * tbl:512 * (tbl + 1)], pc[:])
                nc.sync.dma_start(outT[msl, :], stile[:])
    nc.compile()
    return nc


# de-interleave permutation for rope channels: x1 (even) -> 0:32, x2 (odd) -> 32:64
_PERM = np.concatenate([np.arange(0, ROPE, 2), np.arange(1, ROPE, 2), np.arange(ROPE, hD)])


def prepare_inputs(x, cos, sin, Wq, Wkv, Wo, q_gain):
    """Host-side sharding + layout prep. Returns list of 8 in_maps."""
    x = np.asarray(x, np.float32)
    cos = np.asarray(cos, np.float32)
    sin = np.asarray(sin, np.float32)
    Wq = np.asarray(Wq, np.float32)
    Wkv = np.asarray(Wkv, np.float32)
    Wo = np.asarray(Wo, np.float32)
    q_gain = np.asarray(q_gain, np.float32)

    # rope tables in de-interleaved order: C = [cos; cos], S = [-sin; +sin]
    cosb = np.ascontiguousarray(np.concatenate([cos.T, cos.T], axis=0))   # [64, T]
    sinb = np.ascontiguousarray(np.concatenate([-sin.T, sin.T], axis=0))  # [64, T]

    # additive causal masks for diagonal s-tiles, r = j - 4*ib
    p = np.arange(128)[:, None]
    f = np.arange(512)[None, :]
    maskb = np.zeros((128, 4, 512), np.float32)
    for r in range(4):
        maskb[:, r, :] = np.where(p + 128 * r > f, -1e9, 0.0)

    ident = np.eye(128, dtype=np.float32)
    ones_c = np.ones((128, 1), np.float32)
    ones_r = np.ones((1, 128), np.float32)

    scale = 1.0 / np.sqrt(hD)
    xT = [np.ascontiguousarray(x[b].T) for b in range(B)]

    in_maps = []
    for c in range(8):
        b, hf = divmod(c, 2)
        heads = np.arange(hf * HL, (hf + 1) * HL)
        Wq_h = Wq.reshape(H, hD, D)[heads] * (q_gain[heads, None, None] * scale)
        Wq_h = Wq_h[:, _PERM, :]                                     # de-interleave rope chans
        kvh = np.arange(hf * GL, (hf + 1) * GL)
        Wkv_r = Wkv.reshape(Hkv, 2 * hD, D)[kvh]
        Wk_h = Wkv_r[:, :hD, :][:, _PERM, :]
        Wv_h = Wkv_r[:, hD:, :]
        Wo_h = Wo[:, hf * HL * hD:(hf + 1) * HL * hD]

        in_maps.append({
            "xT": xT[b],
            "wqT": np.ascontiguousarray(Wq_h.reshape(HL * hD, D).T),
            "wkT": np.ascontiguousarray(Wk_h.reshape(GL * hD, D).T),
            "wvT": np.ascontiguousarray(Wv_h.reshape(GL * hD, D).T),
            "woT": np.ascontiguousarray(Wo_h.T),
            "cosb": cosb, "sinb": sinb, "maskb": maskb,
            "ident": ident, "ones_c": ones_c, "ones_r": ones_r,
        })
    return in_maps


_NC_CACHE = {}


def kernel(x, cos, sin, Wq, Wkv, Wo, q_gain, _trace=False):
    if "nc" not in _NC_CACHE:
        _NC_CACHE["nc"] = build_nc()
    nc = _NC_CACHE["nc"]
    in_maps = prepare_inputs(x, cos, sin, Wq, Wkv, Wo, q_gain)
    res = run_bass_kernel_spmd(nc, in_maps, core_ids=list(range(8)), trace=_trace)
    if _trace:
        _NC_CACHE["last_results"] = res
    out = np.empty((B, T, D), np.float32)
    for b in range(B):
        acc = res.results[2 * b]["outT"] + res.results[2 * b + 1]["outT"]
        out[b] = acc.T
    return out


# revision 8
# speedup vs baseline: 1.7712x; 1.0587x over previous
"""Causal self-attention (GQA + partial RoPE + q_gain) Trainium2 Bass kernel.

Model: B=4, T=2048, D=2048, H=16 q-heads, Hkv=4 kv-heads, hD=128, ROPE=64.
Sharding: 8 cores = 4 batches x 2 head-halves (heads hf*8..hf*8+7, kv heads 2hf, 2hf+1).
Wq/Wkv column-sharded, Wo row-sharded; host sums the two partial outputs per batch.

All matmuls fp32r (full PE rate, fp32 storage, tf32-like precision ~1.5e-4).

Per-core dataflow:
  phase A: projections. Q.T/K.T in [channel, token] layout (W stationary, x.T moving),
           V natural [token, channel] (x.T stationary, Wv.T moving). Channels of the
           rope half are de-interleaved host-side (x1 -> 0:32, x2 -> 32:64, consistent
           for Q and K so scores are invariant). RoPE: partner half obtained by two
           32-partition SBUF->SBUF swap DMAs (gpsimd), then 3 aligned DVE ops:
           rope_out[0:64] = qb[0:64]*C + swap*S  with C=[cos;cos], S=[-sin;+sin].
           q_gain and 1/sqrt(hD) folded into Wq host-side. Q.T spilled to DRAM.
  phase B: attention per (head, 512 i-block). Scores TRANSPOSED S.T[s,i] (K.T
           stationary, Q.T moving) so P.T = exp(S.T) feeds AV directly (V natural
           stationary, P.T moving) -> O.T[c,i]. Causal mask: additive -1e9 tiles
           accumulated into scores PSUM via identity-stationary matmul (diagonal
           tiles). Denominators D = ones-stationary (M=1) matmuls on P.T; 1/D via
           fast-approx reciprocal, replicated across partitions by a K=1 matmul;
           O.T normalized during PSUM->SBUF copy.
  phase C: output projection TRANSPOSED: out.T[dout,t] (Wo.T-tile stationary, O.T
           moving). Host adds the two per-batch partials and transposes back.
"""
import numpy as np

import concourse.bass as bass
import concourse.tile as tile
from concourse import bacc, mybir
from concourse.bass_utils import run_bass_kernel_spmd
from contextlib import ExitStack

F32 = mybir.dt.float32
F32R = mybir.dt.float32r
AF = mybir.ActivationFunctionType
AOp = mybir.AluOpType

B, T, D = 4, 2048, 2048
H, Hkv = 16, 4
hD = 128
ROPE = 64
NB = T // 512          # 4 blocks of 512 tokens
HL = H // 2            # 8 heads per core
GL = Hkv // 2          # 2 kv heads per core


def build_nc():
    nc = bacc.Bacc(trn_type="TRN2", target_bir_lowering=False, debug=False)
    xT = nc.dram_tensor("xT", [D, T], F32R, kind="ExternalInput").ap()
    wqT = nc.dram_tensor("wqT", [D, HL * hD], F32R, kind="ExternalInput").ap()
    wkT = nc.dram_tensor("wkT", [D, GL * hD], F32R, kind="ExternalInput").ap()
    wvT = nc.dram_tensor("wvT", [D, GL * hD], F32R, kind="ExternalInput").ap()
    woT = nc.dram_tensor("woT", [HL * hD, D], F32R, kind="ExternalInput").ap()
    cosb = nc.dram_tensor("cosb", [ROPE, T], F32R, kind="ExternalInput").ap()
    sinb = nc.dram_tensor("sinb", [ROPE, T], F32R, kind="ExternalInput").ap()
    maskb = nc.dram_tensor("maskb", [128, 4, 512], F32R, kind="ExternalInput").ap()
    ident = nc.dram_tensor("ident", [128, 128], F32R, kind="ExternalInput").ap()
    ones_c = nc.dram_tensor("ones_c", [128, 1], F32R, kind="ExternalInput").ap()
    ones_r = nc.dram_tensor("ones_r", [1, 128], F32R, kind="ExternalInput").ap()
    outT = nc.dram_tensor("outT", [D, T], F32, kind="ExternalOutput").ap()

    xTr = xT.rearrange("(n p) t -> p n t", p=128)      # [128, 16, 2048]
    wqTr = wqT.rearrange("(n p) m -> p n m", p=128)    # [128, 16, 1024]
    wkTr = wkT.rearrange("(n p) m -> p n m", p=128)    # [128, 16, 256]
    wvTr = wvT.rearrange("(n p) m -> p n m", p=128)

    with tile.TileContext(nc) as tc, ExitStack() as ctx:
        const = ctx.enter_context(tc.tile_pool(name="const", bufs=1))
        persist = ctx.enter_context(tc.tile_pool(name="persist", bufs=1))
        dramp = ctx.enter_context(tc.tile_pool(name="dramp", bufs=1, space="DRAM"))

        tid = const.tile([128, 128], F32R, tag="tid")
        nc.sync.dma_start(tid[:], ident)
        toc = const.tile([128, 1], F32R, tag="toc")
        nc.sync.dma_start(toc[:], ones_c)
        tor = const.tile([1, 128], F32R, tag="tor")
        nc.sync.dma_start(tor[:], ones_r)
        tmask = const.tile([128, 4, 512], F32R, tag="tmask")
        nc.sync.dma_start(tmask[:], maskb)

        QTd = dramp.tile([128, HL, T], F32R, tag="QTd")
        KT = persist.tile([128, GL, T], F32R, tag="KT")
        VT = persist.tile([128, T // 128, GL * hD], F32R, tag="VT")

        # ---------------- phase A: projections + rope ----------------
        with ExitStack() as actx:
            tabs = actx.enter_context(tc.tile_pool(name="tabs", bufs=1))
            xpool = actx.enter_context(tc.tile_pool(name="xp", bufs=17))
            wkvp = actx.enter_context(tc.tile_pool(name="wkvp", bufs=1))
            wqp = actx.enter_context(tc.tile_pool(name="wqp", bufs=2))
            swpool = actx.enter_context(tc.tile_pool(name="swp", bufs=2))
            rtmp = actx.enter_context(tc.tile_pool(name="rtmp", bufs=2))
            qblkp = actx.enter_context(tc.tile_pool(name="qblkp", bufs=3))
            psA = actx.enter_context(tc.tile_pool(name="psA", bufs=1, space="PSUM"))

            tcos = tabs.tile([ROPE, T], F32R, tag="tcos")
            nc.sync.dma_start(tcos[:], cosb)
            tsin = tabs.tile([ROPE, T], F32R, tag="tsin")
            nc.sync.dma_start(tsin[:], sinb)
            # K+V weights resident across all t-blocks (one DMA each)
            wkg = wkvp.tile([128, 16, GL * hD], F32R, tag="wkg", bufs=1, name="wkg")
            nc.sync.dma_start(wkg[:], wkTr)
            wvg = wkvp.tile([128, 16, GL * hD], F32R, tag="wvg", bufs=1, name="wvg")
            nc.sync.dma_start(wvg[:], wvTr)

            for tb in range(NB):
              with nc.named_scope(f"A{tb}"):
                tsl = slice(512 * tb, 512 * (tb + 1))
                xts = []
                for d in range(16):
                    xt = xpool.tile([128, 512], F32R, tag="xt", name=f"xt{tb}_{d}")
                    nc.sync.dma_start(xt[:], xTr[:, d, tsl])
                    xts.append(xt)

                # K projection -> KT[:, g, tsl], then rope via swap-DMA + 3 DVE ops
                for g in range(GL):
                    pk = psA.tile([128, 512], F32, tag="pk", bufs=2, name=f"pk{tb}_{g}")
                    for d in range(16):
                        nc.tensor.matmul(pk[:], wkg[:][:, d, 128 * g:128 * (g + 1)],
                                         xts[d][:], start=(d == 0), stop=(d == 15))
                    nc.vector.tensor_copy(KT[:][:, g, tsl], pk[:])
                    ksw = swpool.tile([ROPE, 512], F32R, tag="ksw", name=f"ksw{tb}_{g}")
                    nc.gpsimd.dma_start(ksw[0:32, :], KT[:][32:64, g, tsl])
                    nc.gpsimd.dma_start(ksw[32:64, :], KT[:][0:32, g, tsl])
                    ts_ = rtmp.tile([ROPE, 512], F32R, tag="ts_", name=f"kts{tb}_{g}")
                    tc_ = rtmp.tile([ROPE, 512], F32R, tag="tc_", name=f"ktc{tb}_{g}")
                    nc.vector.tensor_mul(ts_[:], ksw[:], tsin[:, tsl])
                    nc.vector.tensor_mul(tc_[:], KT[:][0:ROPE, g, tsl], tcos[:, tsl])
                    nc.vector.tensor_tensor(out=KT[:][0:ROPE, g, tsl], in0=tc_[:], in1=ts_[:], op=AOp.add)
                # V projection (natural layout): VT[:, 4*tb+tt, :], tt-groups of 2
                for ttg in range(2):
                    pvs = []
                    for tt in range(2):
                        pv = psA.tile([128, GL * hD], F32, tag=f"pv{tt}", bufs=1,
                                      name=f"pv{tb}_{ttg}_{tt}")
                        pvs.append(pv)
                    for d in range(16):
                        for tt in range(2):
                            tloc = 2 * ttg + tt
                            nc.tensor.matmul(pvs[tt][:], xts[d][:, 128 * tloc:128 * (tloc + 1)],
                                             wvg[:][:, d, :], start=(d == 0), stop=(d == 15))
                    for tt in range(2):
                        nc.vector.tensor_copy(VT[:][:, 4 * tb + 2 * ttg + tt, :], pvs[tt][:])
                # Q projection (m-groups of 2) + rope + spill to DRAM
                for mg in range(HL // 2):
                    wqg = wqp.tile([128, 16, 256], F32R, tag="wqg", name=f"wqg{tb}_{mg}")
                    nc.sync.dma_start(wqg[:], wqTr[:, :, 256 * mg:256 * (mg + 1)])
                    pqs_ = []
                    for mm in range(2):
                        pq = psA.tile([128, 512], F32, tag=f"pq{mm}", bufs=1,
                                      name=f"pq{tb}_{mg}_{mm}")
                        pqs_.append(pq)
                    for d in range(16):
                        for mm in range(2):
                            nc.tensor.matmul(pqs_[mm][:], wqg[:][:, d, 128 * mm:128 * (mm + 1)],
                                             xts[d][:], start=(d == 0), stop=(d == 15))
                    for mm in range(2):
                        h = 2 * mg + mm
                        qb = qblkp.tile([128, 512], F32R, tag="qblk", name=f"qb{tb}_{mg}_{mm}")
                        nc.vector.tensor_copy(qb[:], pqs_[mm][:])
                        qsw = swpool.tile([ROPE, 512], F32R, tag="qsw", name=f"qsw{tb}_{mg}_{mm}")
                        nc.gpsimd.dma_start(qsw[0:32, :], qb[32:64, :])
                        nc.gpsimd.dma_start(qsw[32:64, :], qb[0:32, :])
                        ts_ = rtmp.tile([ROPE, 512], F32R, tag="ts_", name=f"qts{tb}_{mg}_{mm}")
                        tc_ = rtmp.tile([ROPE, 512], F32R, tag="tc_", name=f"qtc{tb}_{mg}_{mm}")
                        nc.vector.tensor_mul(ts_[:], qsw[:], tsin[:, tsl])
                        nc.vector.tensor_mul(tc_[:], qb[0:ROPE, :], tcos[:, tsl])
                        nc.vector.tensor_tensor(out=qb[0:ROPE, :], in0=tc_[:], in1=ts_[:], op=AOp.add)
                        nc.sync.dma_start(QTd[:][:, h, tsl], qb[:])

        # OT pool opened after phase A space is released; lives through B + C
        otp_pool = ctx.enter_context(tc.tile_pool(name="otp_pool", bufs=1))
        OT = otp_pool.tile([128, HL, T], F32R, tag="OT")

        # ---------------- phase B: attention ----------------
        with ExitStack() as bctx:
            ptp = bctx.enter_context(tc.tile_pool(name="ptp", bufs=5))
            qhp = bctx.enter_context(tc.tile_pool(name="qhp", bufs=2))
            nrm = bctx.enter_context(tc.tile_pool(name="nrm", bufs=2))
            psB = bctx.enter_context(tc.tile_pool(name="psB", bufs=1, space="PSUM"))
            for h in range(HL):
              with nc.named_scope(f"B{h}"):
                g = h // (HL // GL)
                qh = qhp.tile([128, T], F32R, tag="qh", name=f"qh{h}")
                nc.sync.dma_start(qh[:], QTd[:][:, h, :])
                for ib in range(NB):
                    isl = slice(512 * ib, 512 * (ib + 1))
                    qsl = qh[:, isl]
                    nj = 4 * ib + 4
                    po = psB.tile([128, 512], F32, tag="po", bufs=2, name=f"po{h}_{ib}")
                    pd = psB.tile([1, 512], F32, tag="pd", bufs=2, name=f"pd{h}_{ib}")
                    for j in range(nj):
                        st = psB.tile([128, 512], F32, tag="st", bufs=3, name=f"st{h}_{ib}_{j}")
                        diag = j >= 4 * ib
                        nc.tensor.matmul(st[:], KT[:][:, g, 128 * j:128 * (j + 1)], qsl,
                                         start=True, stop=not diag)
                        if diag:
                            nc.tensor.matmul(st[:], tid[:], tmask[:][:, j - 4 * ib, :],
                                             start=False, stop=True)
                        pt = ptp.tile([128, 512], F32R, tag="pt", name=f"pt{h}_{ib}_{j}")
                        nc.scalar.activation(pt[:], st[:], AF.Exp)
                        nc.tensor.matmul(po[:], VT[:][:, j, 128 * g:128 * (g + 1)], pt[:],
                                         start=(j == 0), stop=(j == nj - 1))
                        nc.tensor.matmul(pd[:], toc[:], pt[:],
                                         start=(j == 0), stop=(j == nj - 1))
                    dsb = nrm.tile([1, 512], F32, tag="dsb", name=f"dsb{h}_{ib}")
                    nc.vector.tensor_copy(dsb[:], pd[:])
                    rsb = nrm.tile([1, 512], F32, tag="rsb", name=f"rsb{h}_{ib}")
                    nc.vector.reciprocal_approx_fast(rsb[:], dsb[:])
                    rsr = nrm.tile([1, 512], F32R, tag="rsr", name=f"rsr{h}_{ib}")
                    nc.vector.tensor_copy(rsr[:], rsb[:])
                    pr = psB.tile([128, 512], F32, tag="pr", bufs=1, name=f"pr{h}_{ib}")
                    nc.tensor.matmul(pr[:], tor[:], rsr[:], start=True, stop=True)
                    rps = nrm.tile([128, 512], F32, tag="rps", name=f"rps{h}_{ib}")
                    nc.vector.tensor_copy(rps[:], pr[:])
                    nc.vector.tensor_tensor(out=OT[:][:, h, isl], in0=po[:], in1=rps[:], op=AOp.mult)

        # ---------------- phase C: output projection (transposed) ----------------
        with ExitStack() as cctx:
            wop = cctx.enter_context(tc.tile_pool(name="wop", bufs=1))
            stg = cctx.enter_context(tc.tile_pool(name="stg", bufs=2))
            psC = cctx.enter_context(tc.tile_pool(name="psC", bufs=8, space="PSUM"))
            wo_t = []
            for j in range(HL):
                w = wop.tile([128, D], F32R, tag=f"wo{j}", name=f"wo{j}")
                nc.sync.dma_start(w[:], woT[128 * j:128 * (j + 1), :])
                wo_t.append(w)
            for m2 in range(16):
              with nc.named_scope(f"C{m2 // 4}"):
                msl = slice(128 * m2, 128 * (m2 + 1))
                stile = stg.tile([128, T], F32, tag="stile", name=f"stile{m2}")
                for tbl in range(NB):
                    pc = psC.tile([128, 512], F32, tag="pc", name=f"pc{m2}_{tbl}")
                    for j in range(HL):
                        nc.tensor.matmul(pc[:], wo_t[j][:, msl],
                                         OT[:][:, j, 512 * tbl:512 * (tbl + 1)],
                                         start=(j == 0), stop=(j == HL - 1))
                    nc.vector.tensor_copy(stile[:, 512 * tbl:512 * (tbl + 1)], pc[:])
                nc.sync.dma_start(outT[msl, :], stile[:])
    nc.compile()
    return nc


# de-interleave permutation for rope channels: x1 (even) -> 0:32, x2 (odd) -> 32:64
_PERM = np.concatenate([np.arange(0, ROPE, 2), np.arange(1, ROPE, 2), np.arange(ROPE, hD)])


def prepare_inputs(x, cos, sin, Wq, Wkv, Wo, q_gain):
    """Host-side sharding + layout prep. Returns list of 8 in_maps."""
    x = np.asarray(x, np.float32)
    cos = np.asarray(cos, np.float32)
    sin = np.asarray(sin, np.float32)
    Wq = np.asarray(Wq, np.float32)
    Wkv = np.asarray(Wkv, np.float32)
    Wo = np.asarray(Wo, np.float32)
    q_gain = np.asarray(q_gain, np.float32)

    # rope tables in de-interleaved order: C = [cos; cos], S = [-sin; +sin]
    cosb = np.ascontiguousarray(np.concatenate([cos.T, cos.T], axis=0))   # [64, T]
    sinb = np.ascontiguousarray(np.concatenate([-sin.T, sin.T], axis=0))  # [64, T]

    # additive causal masks for diagonal s-tiles, r = j - 4*ib
    p = np.arange(128)[:, None]
    f = np.arange(512)[None, :]
    maskb = np.zeros((128, 4, 512), np.float32)
    for r in range(4):
        maskb[:, r, :] = np.where(p + 128 * r > f, -1e9, 0.0)

    ident = np.eye(128, dtype=np.float32)
    ones_c = np.ones((128, 1), np.float32)
    ones_r = np.ones((1, 128), np.float32)

    scale = 1.0 / np.sqrt(hD)
    xT = [np.ascontiguousarray(x[b].T) for b in range(B)]

    in_maps = []
    for c in range(8):
        b, hf = divmod(c, 2)
        heads = np.arange(hf * HL, (hf + 1) * HL)
        Wq_h = Wq.reshape(H, hD, D)[heads] * (q_gain[heads, None, None] * scale)
        Wq_h = Wq_h[:, _PERM, :]                                     # de-interleave rope chans
        kvh = np.arange(hf * GL, (hf + 1) * GL)
        Wkv_r = Wkv.reshape(Hkv, 2 * hD, D)[kvh]
        Wk_h = Wkv_r[:, :hD, :][:, _PERM, :]
        Wv_h = Wkv_r[:, hD:, :]
        Wo_h = Wo[:, hf * HL * hD:(hf + 1) * HL * hD]

        in_maps.append({
            "xT": xT[b],
            "wqT": np.ascontiguousarray(Wq_h.reshape(HL * hD, D).T),
            "wkT": np.ascontiguousarray(Wk_h.reshape(GL * hD, D).T),
            "wvT": np.ascontiguousarray(Wv_h.reshape(GL * hD, D).T),
            "woT": np.ascontiguousarray(Wo_h.T),
            "cosb": cosb, "sinb": sinb, "maskb": maskb,
            "ident": ident, "ones_c": ones_c, "ones_r": ones_r,
        })
    return in_maps


_NC_CACHE = {}


def kernel(x, cos, sin, Wq, Wkv, Wo, q_gain, _trace=False):
    if "nc" not in _NC_CACHE:
        _NC_CACHE["nc"] = build_nc()
    nc = _NC_CACHE["nc"]
    in_maps = prepare_inputs(x, cos, sin, Wq, Wkv, Wo, q_gain)
    res = run_bass_kernel_spmd(nc, in_maps, core_ids=list(range(8)), trace=_trace)
    if _trace:
        _NC_CACHE["last_results"] = res
    out = np.empty((B, T, D), np.float32)
    for b in range(B):
        acc = res.results[2 * b]["outT"] + res.results[2 * b + 1]["outT"]
        out[b] = acc.T
    return out
